# revision 28
# baseline (speedup 1.0000x reference)
"""Self-contained TRN2 Bass kernel: causal single-head attention.

B=4, S=4096, D=256, fp32 in/out. 8 NeuronCores, data-parallel:
core c = 2*b + h computes batch b, half h of the query blocks
({7,4,3,0} vs {6,5,2,1}). Mixed precision: slots 0-2 (long prefixes)
use fp8e4 DoubleRow matmuls (QK, PV, and a ones-weight matmul for the
softmax denominator); slot 3 (short prefixes, error-sensitive) uses
f32r. No-max softmax with exp bias -2 so P fits fp8 range; denominator
arrives partition-broadcast as lrow[p,q]=l(q), so normalization is a
plain elementwise multiply before the output transpose.
"""

import sys

for _p in ("/opt/trn_rl_repo", "/root/.axon_site/_ro/trn_rl_repo"):
    if _p not in sys.path:
        sys.path.append(_p)

from contextlib import ExitStack

import numpy as np

import concourse.mybir as mybir
import concourse.tile as tile
from concourse import bacc
from concourse.bass_utils import run_bass_kernel_spmd
from concourse.masks import make_identity

F32 = mybir.dt.float32
F32R = mybir.dt.float32r
F8 = mybir.dt.float8e4
BF16 = mybir.dt.bfloat16
DR = mybir.MatmulPerfMode.DoubleRow
A = mybir.AluOpType.add
M = mybir.AluOpType.mult

B, S, D = 4, 4096, 256
NQ = 2048                 # queries per core
NCOUNT = (8, 6, 4, 2)     # key-chunks per slot
SLOTBLK = [[7, 4, 3, 0], [6, 5, 2, 1]]   # abs q-block per slot, per half
SLOT_ORDER = [3, 2, 1, 0]  # shortest-prefix slot first
SCALE = 1.0 / 16.0         # 1/sqrt(D)
BIAS = -2.0                # exp bias (cancels in normalization)


def build():
    nc = bacc.Bacc("TRN2", target_bir_lowering=False, debug=False)
    q8_d = nc.dram_tensor("q8", [128, 3072], F8, kind="ExternalInput").ap()
    k8_d = nc.dram_tensor("k8", [128, 8192], F8, kind="ExternalInput").ap()
    v8_d = nc.dram_tensor("v8", [128, 8192], F8, kind="ExternalInput").ap()
    qf3_d = nc.dram_tensor("qf3", [128, 1024], BF16, kind="ExternalInput").ap()
    kf3_d = nc.dram_tensor("kf3", [128, 2048], BF16, kind="ExternalInput").ap()
    vf3_d = nc.dram_tensor("vf3", [128, 2048], BF16, kind="ExternalInput").ap()
    thr_d = nc.dram_tensor("thr", [128, 32], F32, kind="ExternalInput").ap()
    o_d = nc.dram_tensor("o", [D, NQ], F32, kind="ExternalOutput").ap()

    with tile.TileContext(nc) as tc, ExitStack() as ctx:
        const = ctx.enter_context(tc.tile_pool(name="const", bufs=1))
        sb = ctx.enter_context(tc.tile_pool(name="sb", bufs=8))
        ps = ctx.enter_context(tc.tile_pool(name="ps", bufs=1, space="PSUM"))

        # ---- PE warmup first: ramp the clock during DMA wait ----
        wsrc = const.tile([128, 128], F32R, name="wsrc")
        nc.vector.memset(wsrc[:].bitcast(F32), 1.0)
        for w in range(4):
            wps = ps.tile([128, 512], F32, tag="lrow", bufs=1, name=f"warm{w}")
            nc.tensor.transpose(wps[:, 0:128].bitcast(F32R), wsrc[:], wsrc[:])

        # ---- constant tiles (filled after DMA issue) ----
        ident_f = const.tile([128, 128], F32, name="ident_f")
        ones_f = const.tile([128, 1024], F32, name="ones_f")
        ones8 = const.tile([128, 1024], F8, name="ones8")
        bias_t = const.tile([128, 1], F32, name="bias_t")
        iota = const.tile([128, 1024], F32, name="iota")
        iota_i = const.tile([128, 1024], mybir.dt.int32, name="iota_i")
        thr = const.tile([128, 32], F32, name="thr")

        q8 = const.tile([128, 3072], F8, name="q8")
        k8 = const.tile([128, 8192], F8, name="k8")
        v8 = const.tile([128, 8192], F8, name="v8")
        qf3 = const.tile([128, 1024], BF16, name="qf3")
        kf3 = const.tile([128, 2048], BF16, name="kf3")
        vf3 = const.tile([128, 2048], BF16, name="vf3")

        onesp = ones8[:].rearrange("p (i x) -> p i x", i=2)[:, :, 0:128]

        # ---- input DMAs. DMA bandwidth is globally shared and served
        # roughly FIFO by issue time, so issue strictly in deadline order
        # (slot2 processes items 2,3 first).
        nc.gpsimd.dma_start(out=q8[:, 2048:3072], in_=q8_d[:, 2048:3072])
        nc.sync.dma_start(out=k8[:, 2048:3072], in_=k8_d[:, 2048:3072])
        nc.scalar.dma_start(out=k8[:, 3072:4096], in_=k8_d[:, 3072:4096])
        nc.sync.dma_start(out=v8[:, 2048:3072], in_=v8_d[:, 2048:3072])
        nc.gpsimd.dma_start(out=v8[:, 3072:4096], in_=v8_d[:, 3072:4096])
        nc.sync.dma_start(out=k8[:, 0:2048], in_=k8_d[:, 0:2048])
        nc.scalar.dma_start(out=v8[:, 0:2048], in_=v8_d[:, 0:2048])
        nc.scalar.dma_start(out=thr[:], in_=thr_d)
        nc.gpsimd.dma_start(out=qf3[:], in_=qf3_d)
        nc.gpsimd.dma_start(out=kf3[:], in_=kf3_d)
        nc.scalar.dma_start(out=q8[:, 0:2048], in_=q8_d[:, 0:2048])
        nc.gpsimd.dma_start(out=vf3[:], in_=vf3_d)

        # ---- on-chip constants (after DMA issue so they don't delay it) ----
        make_identity(nc, ident_f[:])
        nc.vector.memset(ones_f[:], 1.0)
        nc.vector.tensor_copy(ones8[:], ones_f[:])
        nc.vector.memset(bias_t[:], BIAS)
        nc.gpsimd.iota(iota_i[:, 0:512], pattern=[[1, 512]], base=0,
                       channel_multiplier=0)
        nc.gpsimd.iota(iota_i[:, 512:1024], pattern=[[1, 512]], base=-128,
                       channel_multiplier=0)
        nc.vector.tensor_copy(iota[:], iota_i[:])

        loaded = [0]

        def load_chunks(upto, first=False):
            # k8/v8 chunks [loaded, upto); one DMA per tensor-half, spread
            # over the three DMA-capable queues (sync/scalar/gpsimd)
            c0 = loaded[0]
            if upto <= c0:
                return
            nc.sync.dma_start(
                out=k8[:, 1024 * c0 : 1024 * upto],
                in_=k8_d[:, 1024 * c0 : 1024 * upto],
            )
            nc.gpsimd.dma_start(
                out=v8[:, 1024 * c0 : 1024 * upto],
                in_=v8_d[:, 1024 * c0 : 1024 * upto],
            )
            loaded[0] = upto

        loaded[0] = 4

        # ---- shared epilogue: recip + scale on DVE, then DMA O^T out
        # directly (host untransposes). No PE involvement at all. ----
        def make_epilogue(st, sO, lrow_t, pre_pe=None):
            def phase1():
                if pre_pe is not None:
                    pre_pe()
                rec = sb.tile([128, 512], F32, tag="rec", bufs=2, name=f"rec{st}")
                nc.vector.reciprocal_approx_fast(out=rec[:], in_=lrow_t[:])
                for dh in range(2):
                    t_ = sb.tile([128, 512], F32, tag="oTf", bufs=4, name=f"oTf{st}{dh}")
                    nc.vector.tensor_tensor(t_[:], sO[dh][:], rec[:], M)
                    nc.sync.dma_start(
                        out=o_d[dh * 128 : (dh + 1) * 128, st * 512 : (st + 1) * 512],
                        in_=t_[:],
                    )

            def phase2():
                pass
            return phase1, phase2

        # ---- fp8 slot (st in {0,1,2}) ----
        def do_slot_fp8(st, epi=None, prefetch_cb=None, fin_prev=None):
            n = NCOUNT[st]
            qx = q8[:, st * 1024 : (st + 1) * 1024].rearrange(
                "p (i x) -> p i x", i=2
            )
            sO = [
                ps.tile([128, 512], F32, tag="sO", bufs=3, name=f"sO{st}{d}")
                for d in range(2)
            ]
            lrow = ps.tile([128, 512], F32, tag="lrow", bufs=1, name=f"lrow{st}")

            def emit_front(t):
                pairs = []
                kc = k8[:, t * 1024 : (t + 1) * 1024].rearrange(
                    "p (i x) -> p i x", i=2
                )
                for pair in range(2):
                    sS = ps.tile([128, 1024], F32, tag="sS", bufs=2, name=f"sS{st}{t}{pair}")
                    for sub in range(2):
                        kt_i = 2 * pair + sub
                        nc.tensor.matmul(
                            sS[:, sub * 512 : (sub + 1) * 512],
                            kc[:, :, 128 * kt_i : 128 * (kt_i + 1)],
                            qx,
                            start=True, stop=True, perf_mode=DR,
                        )
                    p8 = sb.tile([128, 1024], F8, tag="p8", bufs=10, name=f"p8{st}{t}{pair}")
                    nc.scalar.activation(
                        p8[:], sS[:], mybir.ActivationFunctionType.Exp,
                        scale=SCALE, bias=bias_t[:],
                    )
                    pairs.append(p8)
                return pairs

            all_pairs = {}
            # masked (diagonal) items first: their DVE mask latency hides
            # under later items' PE work instead of sitting in the drain
            order = [n - 2, 0, n - 1] + list(range(1, n - 2))

            def emit_lT(j):
                t = order[j]
                for jp01 in range(2):
                    x = all_pairs[t][jp01][:].rearrange("p (i x) -> p i x", i=2)
                    nc.tensor.matmul(
                        lrow[:], onesp, x,
                        start=(j == 0 and jp01 == 0),
                        stop=(j == n - 1 and jp01 == 1),
                        perf_mode=DR,
                    )

            def emit_back(j):
                t = order[j]
                pairs = all_pairs[t]
                if t >= n - 2:
                    pos = t - (n - 2)
                    for pair in range(2):
                        col = st * 8 + pos * 4 + 2 * pair
                        nc.vector.scalar_tensor_tensor(
                            pairs[pair][:],
                            iota[:],
                            thr[:, col : col + 1],
                            pairs[pair][:],
                            mybir.AluOpType.is_ge, M,
                        )
                for jp01 in range(2):
                    jp = 2 * t + jp01
                    x = pairs[jp01][:].rearrange("p (i x) -> p i x", i=2)
                    vv = v8[:, jp * 512 : (jp + 1) * 512].rearrange(
                        "p (i x) -> p i x", i=2
                    )
                    for dh in range(2):
                        nc.tensor.matmul(
                            sO[dh][:], vv[:, :, dh * 128 : (dh + 1) * 128], x,
                            start=(j == 0 and jp01 == 0),
                            stop=(j == n - 1 and jp01 == 1),
                            perf_mode=DR,
                        )
                # lT deferred one item: keeps the bank-clearing first write
                # clear of the previous slot's epilogue
                if j >= 1:
                    emit_lT(j - 1)

            done = [0]
            for pos, t in enumerate(order):
                all_pairs[t] = emit_front(t)
                if pos == 0 and fin_prev is not None:
                    fin_prev[0]()
                if pos == 3 and prefetch_cb is not None:
                    prefetch_cb()
                if pos > 1:
                    emit_back(done[0]); done[0] += 1
                if pos == 1:
                    if fin_prev is not None:
                        fin_prev[1]()
                    if epi is not None:
                        epi[0]()
            while done[0] < n - 2:
                emit_back(done[0]); done[0] += 1

            # last two backs deferred into the next slot so its exps
            # overlap this slot's final PV/lT drain
            def fin1():
                emit_back(done[0]); done[0] += 1

            def fin2():
                emit_back(done[0]); done[0] += 1
                emit_lT(n - 1)
            return (fin1, fin2), make_epilogue(st, sO, lrow)

        # ---- slot 3: f32r path ----
        def do_slot3(epi=None, prefetch_cb=None, fin_prev=None):
            st, n = 3, 2
            sO = [
                ps.tile([128, 512], F32, tag="sO", bufs=3, name=f"sO3{d}")
                for d in range(2)
            ]
            lrow = ps.tile([128, 512], F32, tag="lrow", bufs=1, name="lrow3")
            pSum = sb.tile([128, 512], F32, tag="pSum", bufs=1, name="pSum3")

            def emit_front(t):
                pairs = []
                for pair in range(2):
                    sS = ps.tile([128, 1024], F32, tag="sS", bufs=2, name=f"sS3{t}{pair}")
                    for sub in range(2):
                        kt_i = 2 * pair + sub
                        koff = 512 * t + 128 * kt_i
                        for dt in range(2):
                            nc.tensor.matmul(
                                sS[:, sub * 512 : (sub + 1) * 512],
                                kf3[:, dt * 1024 + koff : dt * 1024 + koff + 128],
                                qf3[:, dt * 512 : (dt + 1) * 512],
                                start=(dt == 0), stop=(dt == 1),
                            )
                    pT = sb.tile([128, 1024], BF16, tag="pT3", bufs=6, name=f"pT3{t}{pair}")
                    nc.scalar.activation(
                        pT[:], sS[:], mybir.ActivationFunctionType.Exp,
                        scale=SCALE, bias=bias_t[:],
                    )
                    pairs.append(pT)
                return pairs

            def emit_back(t, pairs):
                pos = t - (n - 2)
                for pair in range(2):
                    col = st * 8 + pos * 4 + 2 * pair
                    nc.vector.scalar_tensor_tensor(
                        pairs[pair][:],
                        iota[:],
                        thr[:, col : col + 1],
                        pairs[pair][:],
                        mybir.AluOpType.is_ge, M,
                    )
                for kt_i in range(4):
                    pair, sub = kt_i // 2, kt_i % 2
                    g = 4 * t + kt_i
                    x = pairs[pair][:, sub * 512 : (sub + 1) * 512]
                    for dt in range(2):
                        nc.tensor.matmul(
                            sO[dt][:],
                            vf3[:, g * 256 + dt * 128 : g * 256 + (dt + 1) * 128],
                            x,
                            start=(t == 0 and kt_i == 0),
                            stop=(t == n - 1 and kt_i == 3),
                        )
                # denominator partial sums (per-partition)
                f = pairs[0][:]
                g2 = pairs[1][:]
                tmp = sb.tile([128, 512], F32, tag="fold", bufs=2, name=f"fold3{t}")
                nc.vector.tensor_tensor(tmp[:], f[:, 0:512], f[:, 512:1024], A)
                nc.vector.tensor_tensor(tmp[:], tmp[:], g2[:, 0:512], A)
                nc.vector.tensor_tensor(tmp[:], tmp[:], g2[:, 512:1024], A)
                if t == 0:
                    nc.vector.tensor_copy(pSum[:], tmp[:])
                else:
                    nc.vector.tensor_tensor(pSum[:], pSum[:], tmp[:], A)

            hist = {}
            for t in range(n):
                hist[t] = emit_front(t)
                if t == 0 and fin_prev is not None:
                    fin_prev[0]()
                if t == 1 and prefetch_cb is not None:
                    prefetch_cb()
                if t == 1:
                    if fin_prev is not None:
                        fin_prev[1]()
                    if epi is not None:
                        epi[0]()
            pend3 = sorted(hist)

            def fin1():
                emit_back(pend3[0], hist[pend3[0]])

            def fin2():
                emit_back(pend3[1], hist[pend3[1]])
            if epi is not None:
                epi[1]()

            def pre_pe():
                # cross-partition sum via plain f32 ones-matmul (broadcast)
                nc.tensor.matmul(
                    lrow[:], ones_f[:, 0:128], pSum[:], start=True, stop=True
                )
            return (fin1, fin2), make_epilogue(st, sO, lrow, pre_pe=pre_pe)

        # ---- emission ----
        fin, epi = do_slot_fp8(2, prefetch_cb=lambda: load_chunks(6))
        fin, epi = do_slot3(epi=epi, prefetch_cb=lambda: load_chunks(8),
                            fin_prev=fin)
        for st in (1, 0):
            fin, epi = do_slot_fp8(st, epi=epi, fin_prev=fin)
        fin[0]()
        fin[1]()
        epi[0]()
        epi[1]()

    nc.compile()
    return nc


# ---------------- host-side packing ----------------

def make_core_inputs(query, key, value):
    """query/key/value: [B, S, D] f32 numpy. Returns list of 8 in_maps."""
    import ml_dtypes

    f8 = ml_dtypes.float8_e4m3
    kk = np.arange(128, dtype=np.float32)
    in_maps = []
    per_batch = {}
    for b in range(B):
        K8 = key[b].astype(f8)
        V8 = value[b].astype(f8)
        # chunk-interleaved: [p, c*1024 + i*512 + k]
        k8 = np.zeros((128, 8192), dtype=K8.dtype)
        for c in range(8):
            for i in range(2):
                k8[:, c * 1024 + i * 512 : c * 1024 + (i + 1) * 512] = K8[
                    512 * c : 512 * (c + 1), 128 * i : 128 * (i + 1)
                ].T
        v8 = (
            V8.reshape(16, 2, 128, 256).transpose(2, 0, 1, 3).reshape(128, 8192)
        )
        kf3 = np.concatenate(
            [key[b, :1024, :128].T, key[b, :1024, 128:].T], axis=1
        ).astype(ml_dtypes.bfloat16)  # [128, 2048]
        vf3 = (
            value[b, :1024].reshape(8, 128, 256).transpose(1, 0, 2).reshape(128, 2048)
        ).astype(ml_dtypes.bfloat16)
        per_batch[b] = (k8, v8, kf3, vf3)

    for c in range(8):
        b, h = c // 2, c % 2
        blocks = SLOTBLK[h]
        k8, v8, kf3, vf3 = per_batch[b]
        # q8: slots 0..2, transposed pair-packed fp8
        q8 = np.zeros((128, 3072), dtype=np.float32)
        for st in range(3):
            blk = blocks[st]
            Qb = query[b, 512 * blk : 512 * (blk + 1)]  # [512, 256]
            for i in range(2):
                q8[:, st * 1024 + i * 512 : st * 1024 + (i + 1) * 512] = Qb[
                    :, i * 128 : (i + 1) * 128
                ].T
        q8 = q8.astype(ml_dtypes.float8_e4m3)
        # qf3: slot3 block, f32 transposed
        blk3 = blocks[3]
        Q3 = query[b, 512 * blk3 : 512 * (blk3 + 1)]
        qf3 = np.concatenate([Q3[:, :128].T, Q3[:, 128:].T], axis=1).astype(
            ml_dtypes.bfloat16
        )
        thr = np.zeros((128, 32), dtype=np.float32)
        for st in range(4):
            j_abs = blocks[st]
            n = NCOUNT[st]
            for pos in range(2):
                chunk = n - 2 + pos
                for kt in range(4):
                    col = st * 8 + pos * 4 + kt
                    if chunk < j_abs:
                        thr[:, col] = -1e4
                    elif chunk == j_abs:
                        thr[:, col] = 128.0 * kt + kk
                    else:
                        thr[:, col] = 1e4
        in_maps.append(
            {
                "q8": np.ascontiguousarray(q8),
                "k8": np.ascontiguousarray(k8),
                "v8": np.ascontiguousarray(v8),
                "qf3": np.ascontiguousarray(qf3),
                "kf3": np.ascontiguousarray(kf3),
                "vf3": np.ascontiguousarray(vf3),
                "thr": thr,
            }
        )
    return in_maps


def gather_output(results):
    """results: list of 8 dicts with 'o' [NQ, D]. Returns [B, S, D]."""
    out = np.zeros((B, S, D), dtype=np.float32)
    for c in range(8):
        b, h = c // 2, c % 2
        o = results[c]["o"]  # [D, NQ] transposed
        for st, blk in enumerate(SLOTBLK[h]):
            out[b, 512 * blk : 512 * (blk + 1)] = o[:, 512 * st : 512 * (st + 1)].T
    return out


_NC_CACHE = []


def kernel(query, key, value, attention_mask):
    """Full-input causal attention; returns [B, S, D] float32."""
    query = np.ascontiguousarray(np.asarray(query, dtype=np.float32))
    key = np.ascontiguousarray(np.asarray(key, dtype=np.float32))
    value = np.ascontiguousarray(np.asarray(value, dtype=np.float32))
    assert query.shape == (B, S, D) and key.shape == (B, S, D)
    assert value.shape == (B, S, D)
    # attention_mask is all-ones by problem construction (fill: ones).
    if not _NC_CACHE:
        _NC_CACHE.append(build())
    nc = _NC_CACHE[0]
    in_maps = make_core_inputs(query, key, value)
    res = run_bass_kernel_spmd(nc, in_maps, core_ids=list(range(8)))
    return gather_output(res.results)


# revision 29
# speedup vs baseline: 1.1610x; 1.1610x over previous
"""Self-contained TRN2 Bass kernel: causal single-head attention.

B=4, S=4096, D=256, fp32 in/out. 8 NeuronCores, data-parallel:
core c = 2*b + h computes batch b, half h of the query blocks
({7,4,3,0} vs {6,5,2,1}). Mixed precision: slots 0-2 (long prefixes)
use fp8e4 DoubleRow matmuls (QK, PV, and a ones-weight matmul for the
softmax denominator); slot 3 (short prefixes, error-sensitive) uses
f32r. No-max softmax with exp bias -2 so P fits fp8 range; denominator
arrives partition-broadcast as lrow[p,q]=l(q), so normalization is a
plain elementwise multiply before the output transpose.
"""

import sys

for _p in ("/opt/trn_rl_repo", "/root/.axon_site/_ro/trn_rl_repo"):
    if _p not in sys.path:
        sys.path.append(_p)

from contextlib import ExitStack

import numpy as np

import concourse.mybir as mybir
import concourse.tile as tile
from concourse import bacc
from concourse.bass_utils import run_bass_kernel_spmd
from concourse.masks import make_identity

F32 = mybir.dt.float32
F32R = mybir.dt.float32r
F8 = mybir.dt.float8e4
BF16 = mybir.dt.bfloat16
DR = mybir.MatmulPerfMode.DoubleRow
A = mybir.AluOpType.add
M = mybir.AluOpType.mult

B, S, D = 4, 4096, 256
NQ = 2048                 # queries per core
NCOUNT = (8, 6, 4, 2)     # key-chunks per slot
SLOTBLK = [[7, 4, 3, 0], [6, 5, 2, 1]]   # abs q-block per slot, per half
SLOT_ORDER = [3, 2, 1, 0]  # shortest-prefix slot first
SCALE = 1.0 / 16.0         # 1/sqrt(D)
BIAS = -2.0                # exp bias (cancels in normalization)


def build():
    nc = bacc.Bacc("TRN2", target_bir_lowering=False, debug=False)
    q8_d = nc.dram_tensor("q8", [128, 3072], F8, kind="ExternalInput").ap()
    k8_d = nc.dram_tensor("k8", [128, 8192], F8, kind="ExternalInput").ap()
    v8_d = nc.dram_tensor("v8", [128, 8192], F8, kind="ExternalInput").ap()
    qf3_d = nc.dram_tensor("qf3", [128, 1024], BF16, kind="ExternalInput").ap()
    kf3_d = nc.dram_tensor("kf3", [128, 2048], BF16, kind="ExternalInput").ap()
    vf3_d = nc.dram_tensor("vf3", [128, 2048], BF16, kind="ExternalInput").ap()
    thr_d = nc.dram_tensor("thr", [128, 32], F32, kind="ExternalInput").ap()
    o_d = nc.dram_tensor("o", [D, NQ], F32, kind="ExternalOutput").ap()

    with tile.TileContext(nc) as tc, ExitStack() as ctx:
        const = ctx.enter_context(tc.tile_pool(name="const", bufs=1))
        sb = ctx.enter_context(tc.tile_pool(name="sb", bufs=8))
        ps = ctx.enter_context(tc.tile_pool(name="ps", bufs=1, space="PSUM"))

        # ---- PE warmup first: ramp the clock during DMA wait ----
        wsrc = const.tile([128, 128], F32R, name="wsrc")
        nc.vector.memset(wsrc[:].bitcast(F32), 1.0)
        for w in range(4):
            wps = ps.tile([128, 512], F32, tag="lrow", bufs=1, name=f"warm{w}")
            nc.tensor.transpose(wps[:, 0:128].bitcast(F32R), wsrc[:], wsrc[:])

        # ---- constant tiles (filled after DMA issue) ----
        ident_f = const.tile([128, 128], F32, name="ident_f")
        ones_f = const.tile([128, 1024], F32, name="ones_f")
        ones8 = const.tile([128, 1024], F8, name="ones8")
        bias_t = const.tile([128, 1], F32, name="bias_t")
        iota = const.tile([128, 1024], F32, name="iota")
        iota_i = const.tile([128, 1024], mybir.dt.int32, name="iota_i")
        thr = const.tile([128, 32], F32, name="thr")

        q8 = const.tile([128, 3072], F8, name="q8")
        k8 = const.tile([128, 8192], F8, name="k8")
        v8 = const.tile([128, 8192], F8, name="v8")
        qf3 = const.tile([128, 1024], BF16, name="qf3")
        kf3 = const.tile([128, 2048], BF16, name="kf3")
        vf3 = const.tile([128, 2048], BF16, name="vf3")

        onesp = ones8[:].rearrange("p (i x) -> p i x", i=2)[:, :, 0:128]

        # ---- input DMAs. DMA bandwidth is globally shared and served
        # roughly FIFO by issue time, so issue strictly in deadline order
        # (slot2 processes items 2,3 first).
        # processing order within slot2 is items [2, 0, 3, 1]
        nc.gpsimd.dma_start(out=q8[:, 2048:3072], in_=q8_d[:, 2048:3072])
        nc.sync.dma_start(out=k8[:, 2048:3072], in_=k8_d[:, 2048:3072])
        nc.scalar.dma_start(out=k8[:, 0:1024], in_=k8_d[:, 0:1024])
        nc.sync.dma_start(out=v8[:, 2048:3072], in_=v8_d[:, 2048:3072])
        nc.gpsimd.dma_start(out=k8[:, 3072:4096], in_=k8_d[:, 3072:4096])
        nc.scalar.dma_start(out=v8[:, 0:1024], in_=v8_d[:, 0:1024])
        nc.sync.dma_start(out=k8[:, 1024:2048], in_=k8_d[:, 1024:2048])
        nc.gpsimd.dma_start(out=v8[:, 3072:4096], in_=v8_d[:, 3072:4096])
        nc.scalar.dma_start(out=v8[:, 1024:2048], in_=v8_d[:, 1024:2048])
        nc.sync.dma_start(out=thr[:], in_=thr_d)
        nc.gpsimd.dma_start(out=qf3[:], in_=qf3_d)
        nc.sync.dma_start(out=kf3[:], in_=kf3_d)
        nc.scalar.dma_start(out=q8[:, 0:2048], in_=q8_d[:, 0:2048])
        nc.gpsimd.dma_start(out=vf3[:], in_=vf3_d)

        # ---- on-chip constants (after DMA issue so they don't delay it) ----
        make_identity(nc, ident_f[:])
        nc.vector.memset(ones_f[:], 1.0)
        nc.vector.tensor_copy(ones8[:], ones_f[:])
        nc.vector.memset(bias_t[:], BIAS)
        nc.gpsimd.iota(iota_i[:, 0:512], pattern=[[1, 512]], base=0,
                       channel_multiplier=0)
        nc.gpsimd.iota(iota_i[:, 512:1024], pattern=[[1, 512]], base=-128,
                       channel_multiplier=0)
        nc.vector.tensor_copy(iota[:], iota_i[:])

        loaded = [0]

        def load_chunks(upto, first=False):
            # k8/v8 chunks [loaded, upto); one DMA per tensor-half, spread
            # over the three DMA-capable queues (sync/scalar/gpsimd)
            c0 = loaded[0]
            if upto <= c0:
                return
            nc.sync.dma_start(
                out=k8[:, 1024 * c0 : 1024 * upto],
                in_=k8_d[:, 1024 * c0 : 1024 * upto],
            )
            nc.gpsimd.dma_start(
                out=v8[:, 1024 * c0 : 1024 * upto],
                in_=v8_d[:, 1024 * c0 : 1024 * upto],
            )
            loaded[0] = upto

        loaded[0] = 4

        # ---- shared epilogue: recip + scale on DVE, then DMA O^T out
        # directly (host untransposes). No PE involvement at all. ----
        def make_epilogue(st, sO, lrow_t, pre_pe=None):
            def phase1():
                if pre_pe is not None:
                    pre_pe()
                rec = sb.tile([128, 512], F32, tag="rec", bufs=2, name=f"rec{st}")
                nc.vector.reciprocal_approx_fast(out=rec[:], in_=lrow_t[:])
                for dh in range(2):
                    t_ = sb.tile([128, 512], F32, tag="oTf", bufs=4, name=f"oTf{st}{dh}")
                    nc.vector.tensor_tensor(t_[:], sO[dh][:], rec[:], M)
                    nc.sync.dma_start(
                        out=o_d[dh * 128 : (dh + 1) * 128, st * 512 : (st + 1) * 512],
                        in_=t_[:],
                    )

            def phase2():
                pass
            return phase1, phase2

        # ---- fp8 slot (st in {0,1,2}) ----
        def do_slot_fp8(st, epi=None, prefetch_cb=None, fin_prev=None):
            n = NCOUNT[st]
            qx = q8[:, st * 1024 : (st + 1) * 1024].rearrange(
                "p (i x) -> p i x", i=2
            )
            sO = [
                ps.tile([128, 512], F32, tag="sO", bufs=3, name=f"sO{st}{d}")
                for d in range(2)
            ]
            lrow = ps.tile([128, 512], F32, tag="lrow", bufs=1, name=f"lrow{st}")

            def emit_front(t):
                pairs = []
                kc = k8[:, t * 1024 : (t + 1) * 1024].rearrange(
                    "p (i x) -> p i x", i=2
                )
                for pair in range(2):
                    sS = ps.tile([128, 1024], F32, tag="sS", bufs=2, name=f"sS{st}{t}{pair}")
                    for sub in range(2):
                        kt_i = 2 * pair + sub
                        nc.tensor.matmul(
                            sS[:, sub * 512 : (sub + 1) * 512],
                            kc[:, :, 128 * kt_i : 128 * (kt_i + 1)],
                            qx,
                            start=True, stop=True, perf_mode=DR,
                        )
                    p8 = sb.tile([128, 1024], F8, tag="p8", bufs=10, name=f"p8{st}{t}{pair}")
                    nc.scalar.activation(
                        p8[:], sS[:], mybir.ActivationFunctionType.Exp,
                        scale=SCALE, bias=bias_t[:],
                    )
                    pairs.append(p8)
                return pairs

            all_pairs = {}
            # masked (diagonal) items first: their DVE mask latency hides
            # under later items' PE work instead of sitting in the drain
            order = [n - 2, 0, n - 1] + list(range(1, n - 2))

            def emit_lT(j):
                t = order[j]
                for jp01 in range(2):
                    x = all_pairs[t][jp01][:].rearrange("p (i x) -> p i x", i=2)
                    nc.tensor.matmul(
                        lrow[:], onesp, x,
                        start=(j == 0 and jp01 == 0),
                        stop=(j == n - 1 and jp01 == 1),
                        perf_mode=DR,
                    )

            def emit_back(j):
                t = order[j]
                pairs = all_pairs[t]
                if t >= n - 2:
                    pos = t - (n - 2)
                    for pair in range(2):
                        col = st * 8 + pos * 4 + 2 * pair
                        nc.vector.scalar_tensor_tensor(
                            pairs[pair][:],
                            iota[:],
                            thr[:, col : col + 1],
                            pairs[pair][:],
                            mybir.AluOpType.is_ge, M,
                        )
                for jp01 in range(2):
                    jp = 2 * t + jp01
                    x = pairs[jp01][:].rearrange("p (i x) -> p i x", i=2)
                    vv = v8[:, jp * 512 : (jp + 1) * 512].rearrange(
                        "p (i x) -> p i x", i=2
                    )
                    for dh in range(2):
                        nc.tensor.matmul(
                            sO[dh][:], vv[:, :, dh * 128 : (dh + 1) * 128], x,
                            start=(j == 0 and jp01 == 0),
                            stop=(j == n - 1 and jp01 == 1),
                            perf_mode=DR,
                        )
                # lT deferred one item: keeps the bank-clearing first write
                # clear of the previous slot's epilogue
                if j >= 1:
                    emit_lT(j - 1)

            done = [0]
            for pos, t in enumerate(order):
                all_pairs[t] = emit_front(t)
                if pos == 0 and fin_prev is not None:
                    fin_prev[0]()
                if pos == 3 and prefetch_cb is not None:
                    prefetch_cb()
                if pos > 1:
                    emit_back(done[0]); done[0] += 1
                if pos == 1:
                    if fin_prev is not None:
                        fin_prev[1]()
                    if epi is not None:
                        epi[0]()
            while done[0] < n - 2:
                emit_back(done[0]); done[0] += 1

            # last two backs deferred into the next slot so its exps
            # overlap this slot's final PV/lT drain
            def fin1():
                emit_back(done[0]); done[0] += 1

            def fin2():
                emit_back(done[0]); done[0] += 1
                emit_lT(n - 1)
            return (fin1, fin2), make_epilogue(st, sO, lrow)

        # ---- slot 3: f32r path ----
        def do_slot3(epi=None, prefetch_cb=None, fin_prev=None):
            st, n = 3, 2
            sO = [
                ps.tile([128, 512], F32, tag="sO", bufs=3, name=f"sO3{d}")
                for d in range(2)
            ]
            lrow = ps.tile([128, 512], F32, tag="lrow", bufs=1, name="lrow3")
            pSum = sb.tile([128, 512], F32, tag="pSum", bufs=1, name="pSum3")

            def emit_front(t):
                pairs = []
                for pair in range(2):
                    sS = ps.tile([128, 1024], F32, tag="sS", bufs=2, name=f"sS3{t}{pair}")
                    for sub in range(2):
                        kt_i = 2 * pair + sub
                        koff = 512 * t + 128 * kt_i
                        for dt in range(2):
                            nc.tensor.matmul(
                                sS[:, sub * 512 : (sub + 1) * 512],
                                kf3[:, dt * 1024 + koff : dt * 1024 + koff + 128],
                                qf3[:, dt * 512 : (dt + 1) * 512],
                                start=(dt == 0), stop=(dt == 1),
                            )
                    pT = sb.tile([128, 1024], BF16, tag="pT3", bufs=6, name=f"pT3{t}{pair}")
                    nc.scalar.activation(
                        pT[:], sS[:], mybir.ActivationFunctionType.Exp,
                        scale=SCALE, bias=bias_t[:],
                    )
                    pairs.append(pT)
                return pairs

            def emit_back(t, pairs):
                pos = t - (n - 2)
                for pair in range(2):
                    col = st * 8 + pos * 4 + 2 * pair
                    nc.vector.scalar_tensor_tensor(
                        pairs[pair][:],
                        iota[:],
                        thr[:, col : col + 1],
                        pairs[pair][:],
                        mybir.AluOpType.is_ge, M,
                    )
                for kt_i in range(4):
                    pair, sub = kt_i // 2, kt_i % 2
                    g = 4 * t + kt_i
                    x = pairs[pair][:, sub * 512 : (sub + 1) * 512]
                    for dt in range(2):
                        nc.tensor.matmul(
                            sO[dt][:],
                            vf3[:, g * 256 + dt * 128 : g * 256 + (dt + 1) * 128],
                            x,
                            start=(t == 0 and kt_i == 0),
                            stop=(t == n - 1 and kt_i == 3),
                        )
                # denominator partial sums (per-partition)
                f = pairs[0][:]
                g2 = pairs[1][:]
                tmp = sb.tile([128, 512], F32, tag="fold", bufs=2, name=f"fold3{t}")
                nc.vector.tensor_tensor(tmp[:], f[:, 0:512], f[:, 512:1024], A)
                nc.vector.tensor_tensor(tmp[:], tmp[:], g2[:, 0:512], A)
                nc.vector.tensor_tensor(tmp[:], tmp[:], g2[:, 512:1024], A)
                if t == 0:
                    nc.vector.tensor_copy(pSum[:], tmp[:])
                else:
                    nc.vector.tensor_tensor(pSum[:], pSum[:], tmp[:], A)

            hist = {}
            for t in range(n):
                hist[t] = emit_front(t)
                if t == 0 and fin_prev is not None:
                    fin_prev[0]()
                if t == 1 and prefetch_cb is not None:
                    prefetch_cb()
                if t == 1:
                    if fin_prev is not None:
                        fin_prev[1]()
                    if epi is not None:
                        epi[0]()
            pend3 = sorted(hist)

            def fin1():
                emit_back(pend3[0], hist[pend3[0]])

            def fin2():
                emit_back(pend3[1], hist[pend3[1]])
            if epi is not None:
                epi[1]()

            def pre_pe():
                # cross-partition sum via plain f32 ones-matmul (broadcast)
                nc.tensor.matmul(
                    lrow[:], ones_f[:, 0:128], pSum[:], start=True, stop=True
                )
            return (fin1, fin2), make_epilogue(st, sO, lrow, pre_pe=pre_pe)

        # ---- emission ----
        fin, epi = do_slot_fp8(2, prefetch_cb=lambda: load_chunks(6))
        fin, epi = do_slot3(epi=epi, prefetch_cb=lambda: load_chunks(8),
                            fin_prev=fin)
        for st in (1, 0):
            fin, epi = do_slot_fp8(st, epi=epi, fin_prev=fin)
        fin[0]()
        fin[1]()
        epi[0]()
        epi[1]()

    nc.compile()
    return nc


# ---------------- host-side packing ----------------

def make_core_inputs(query, key, value):
    """query/key/value: [B, S, D] f32 numpy. Returns list of 8 in_maps."""
    import ml_dtypes

    f8 = ml_dtypes.float8_e4m3
    kk = np.arange(128, dtype=np.float32)
    in_maps = []
    per_batch = {}
    for b in range(B):
        K8 = key[b].astype(f8)
        V8 = value[b].astype(f8)
        # chunk-interleaved: [p, c*1024 + i*512 + k]
        k8 = np.zeros((128, 8192), dtype=K8.dtype)
        for c in range(8):
            for i in range(2):
                k8[:, c * 1024 + i * 512 : c * 1024 + (i + 1) * 512] = K8[
                    512 * c : 512 * (c + 1), 128 * i : 128 * (i + 1)
                ].T
        v8 = (
            V8.reshape(16, 2, 128, 256).transpose(2, 0, 1, 3).reshape(128, 8192)
        )
        kf3 = np.concatenate(
            [key[b, :1024, :128].T, key[b, :1024, 128:].T], axis=1
        ).astype(ml_dtypes.bfloat16)  # [128, 2048]
        vf3 = (
            value[b, :1024].reshape(8, 128, 256).transpose(1, 0, 2).reshape(128, 2048)
        ).astype(ml_dtypes.bfloat16)
        per_batch[b] = (k8, v8, kf3, vf3)

    for c in range(8):
        b, h = c // 2, c % 2
        blocks = SLOTBLK[h]
        k8, v8, kf3, vf3 = per_batch[b]
        # q8: slots 0..2, transposed pair-packed fp8
        q8 = np.zeros((128, 3072), dtype=np.float32)
        for st in range(3):
            blk = blocks[st]
            Qb = query[b, 512 * blk : 512 * (blk + 1)]  # [512, 256]
            for i in range(2):
                q8[:, st * 1024 + i * 512 : st * 1024 + (i + 1) * 512] = Qb[
                    :, i * 128 : (i + 1) * 128
                ].T
        q8 = q8.astype(ml_dtypes.float8_e4m3)
        # qf3: slot3 block, f32 transposed
        blk3 = blocks[3]
        Q3 = query[b, 512 * blk3 : 512 * (blk3 + 1)]
        qf3 = np.concatenate([Q3[:, :128].T, Q3[:, 128:].T], axis=1).astype(
            ml_dtypes.bfloat16
        )
        thr = np.zeros((128, 32), dtype=np.float32)
        for st in range(4):
            j_abs = blocks[st]
            n = NCOUNT[st]
            for pos in range(2):
                chunk = n - 2 + pos
                for kt in range(4):
                    col = st * 8 + pos * 4 + kt
                    if chunk < j_abs:
                        thr[:, col] = -1e4
                    elif chunk == j_abs:
                        thr[:, col] = 128.0 * kt + kk
                    else:
                        thr[:, col] = 1e4
        in_maps.append(
            {
                "q8": np.ascontiguousarray(q8),
                "k8": np.ascontiguousarray(k8),
                "v8": np.ascontiguousarray(v8),
                "qf3": np.ascontiguousarray(qf3),
                "kf3": np.ascontiguousarray(kf3),
                "vf3": np.ascontiguousarray(vf3),
                "thr": thr,
            }
        )
    return in_maps


def gather_output(results):
    """results: list of 8 dicts with 'o' [NQ, D]. Returns [B, S, D]."""
    out = np.zeros((B, S, D), dtype=np.float32)
    for c in range(8):
        b, h = c // 2, c % 2
        o = results[c]["o"]  # [D, NQ] transposed
        for st, blk in enumerate(SLOTBLK[h]):
            out[b, 512 * blk : 512 * (blk + 1)] = o[:, 512 * st : 512 * (st + 1)].T
    return out


_NC_CACHE = []


def kernel(query, key, value, attention_mask):
    """Full-input causal attention; returns [B, S, D] float32."""
    query = np.ascontiguousarray(np.asarray(query, dtype=np.float32))
    key = np.ascontiguousarray(np.asarray(key, dtype=np.float32))
    value = np.ascontiguousarray(np.asarray(value, dtype=np.float32))
    assert query.shape == (B, S, D) and key.shape == (B, S, D)
    assert value.shape == (B, S, D)
    # attention_mask is all-ones by problem construction (fill: ones).
    if not _NC_CACHE:
        _NC_CACHE.append(build())
    nc = _NC_CACHE[0]
    in_maps = make_core_inputs(query, key, value)
    res = run_bass_kernel_spmd(nc, in_maps, core_ids=list(range(8)))
    return gather_output(res.results)


# revision 30
# speedup vs baseline: 1.1682x; 1.0062x over previous
"""Self-contained TRN2 Bass kernel: causal single-head attention.

B=4, S=4096, D=256, fp32 in/out. 8 NeuronCores, data-parallel:
core c = 2*b + h computes batch b, half h of the query blocks
({7,4,3,0} vs {6,5,2,1}). Mixed precision: slots 0-2 (long prefixes)
use fp8e4 DoubleRow matmuls (QK, PV, and a ones-weight matmul for the
softmax denominator); slot 3 (short prefixes, error-sensitive) uses
f32r. No-max softmax with exp bias -2 so P fits fp8 range; denominator
arrives partition-broadcast as lrow[p,q]=l(q), so normalization is a
plain elementwise multiply before the output transpose.
"""

import sys

for _p in ("/opt/trn_rl_repo", "/root/.axon_site/_ro/trn_rl_repo"):
    if _p not in sys.path:
        sys.path.append(_p)

from contextlib import ExitStack

import numpy as np

import concourse.mybir as mybir
import concourse.tile as tile
from concourse import bacc
from concourse.bass_utils import run_bass_kernel_spmd
from concourse.masks import make_identity

F32 = mybir.dt.float32
F32R = mybir.dt.float32r
F8 = mybir.dt.float8e4
BF16 = mybir.dt.bfloat16
DR = mybir.MatmulPerfMode.DoubleRow
A = mybir.AluOpType.add
M = mybir.AluOpType.mult

B, S, D = 4, 4096, 256
NQ = 2048                 # queries per core
NCOUNT = (8, 6, 4, 2)     # key-chunks per slot
SLOTBLK = [[7, 4, 3, 0], [6, 5, 2, 1]]   # abs q-block per slot, per half
SLOT_ORDER = [3, 2, 1, 0]  # shortest-prefix slot first
SCALE = 1.0 / 16.0         # 1/sqrt(D)
BIAS = -2.0                # exp bias (cancels in normalization)


def build():
    nc = bacc.Bacc("TRN2", target_bir_lowering=False, debug=False)
    q8_d = nc.dram_tensor("q8", [128, 3072], F8, kind="ExternalInput").ap()
    k8_d = nc.dram_tensor("k8", [128, 8192], F8, kind="ExternalInput").ap()
    v8_d = nc.dram_tensor("v8", [128, 8192], F8, kind="ExternalInput").ap()
    qf3_d = nc.dram_tensor("qf3", [128, 1024], BF16, kind="ExternalInput").ap()
    kf3_d = nc.dram_tensor("kf3", [128, 2048], BF16, kind="ExternalInput").ap()
    vf3_d = nc.dram_tensor("vf3", [128, 2048], BF16, kind="ExternalInput").ap()
    thr_d = nc.dram_tensor("thr", [128, 32], F32, kind="ExternalInput").ap()
    o_d = nc.dram_tensor("o", [D, NQ], F32, kind="ExternalOutput").ap()

    with tile.TileContext(nc) as tc, ExitStack() as ctx:
        const = ctx.enter_context(tc.tile_pool(name="const", bufs=1))
        sb = ctx.enter_context(tc.tile_pool(name="sb", bufs=8))
        ps = ctx.enter_context(tc.tile_pool(name="ps", bufs=1, space="PSUM"))

        # ---- PE warmup first: ramp the clock during DMA wait ----
        wsrc = const.tile([128, 128], F32R, name="wsrc")
        nc.vector.memset(wsrc[:].bitcast(F32), 1.0)
        for w in range(4):
            wps = ps.tile([128, 512], F32, tag="lrow", bufs=1, name=f"warm{w}")
            nc.tensor.transpose(wps[:, 0:128].bitcast(F32R), wsrc[:], wsrc[:])

        # ---- constant tiles (filled after DMA issue) ----
        ident_f = const.tile([128, 128], F32, name="ident_f")
        ones_f = const.tile([128, 1024], F32, name="ones_f")
        ones8 = const.tile([128, 1024], F8, name="ones8")
        bias_t = const.tile([128, 1], F32, name="bias_t")
        iota = const.tile([128, 1024], F32, name="iota")
        iota_i = const.tile([128, 1024], mybir.dt.int32, name="iota_i")
        thr = const.tile([128, 32], F32, name="thr")

        q8 = const.tile([128, 3072], F8, name="q8")
        k8 = const.tile([128, 8192], F8, name="k8")
        v8 = const.tile([128, 8192], F8, name="v8")
        qf3 = const.tile([128, 1024], BF16, name="qf3")
        kf3 = const.tile([128, 2048], BF16, name="kf3")
        vf3 = const.tile([128, 2048], BF16, name="vf3")

        onesp = ones8[:].rearrange("p (i x) -> p i x", i=2)[:, :, 0:128]

        # ---- input DMAs. DMA bandwidth is globally shared and served
        # roughly FIFO by issue time, so issue strictly in deadline order
        # (slot2 processes items 2,3 first).
        # processing order within slot2 is items [2, 0, 3, 1]
        # fronts consume k-chunks 2 positions before backs consume the
        # matching v-chunks; interleave deadlines: k2,k0,k3,v2,k1,v0,v3,v1
        nc.gpsimd.dma_start(out=q8[:, 2048:3072], in_=q8_d[:, 2048:3072])
        nc.sync.dma_start(out=k8[:, 2048:3072], in_=k8_d[:, 2048:3072])
        nc.scalar.dma_start(out=k8[:, 0:1024], in_=k8_d[:, 0:1024])
        nc.sync.dma_start(out=k8[:, 3072:4096], in_=k8_d[:, 3072:4096])
        nc.gpsimd.dma_start(out=v8[:, 2048:3072], in_=v8_d[:, 2048:3072])
        nc.scalar.dma_start(out=k8[:, 1024:2048], in_=k8_d[:, 1024:2048])
        nc.sync.dma_start(out=v8[:, 0:1024], in_=v8_d[:, 0:1024])
        nc.gpsimd.dma_start(out=v8[:, 3072:4096], in_=v8_d[:, 3072:4096])
        nc.scalar.dma_start(out=v8[:, 1024:2048], in_=v8_d[:, 1024:2048])
        nc.sync.dma_start(out=thr[:], in_=thr_d)
        nc.gpsimd.dma_start(out=qf3[:], in_=qf3_d)
        nc.sync.dma_start(out=kf3[:], in_=kf3_d)
        nc.scalar.dma_start(out=q8[:, 0:2048], in_=q8_d[:, 0:2048])
        nc.gpsimd.dma_start(out=vf3[:], in_=vf3_d)

        # ---- on-chip constants (after DMA issue so they don't delay it) ----
        make_identity(nc, ident_f[:])
        nc.vector.memset(ones_f[:], 1.0)
        nc.vector.tensor_copy(ones8[:], ones_f[:])
        nc.vector.memset(bias_t[:], BIAS)
        nc.gpsimd.iota(iota_i[:, 0:512], pattern=[[1, 512]], base=0,
                       channel_multiplier=0)
        nc.gpsimd.iota(iota_i[:, 512:1024], pattern=[[1, 512]], base=-128,
                       channel_multiplier=0)
        nc.vector.tensor_copy(iota[:], iota_i[:])

        loaded = [0]

        def load_chunks(upto, first=False):
            # k8/v8 chunks [loaded, upto); one DMA per tensor-half, spread
            # over the three DMA-capable queues (sync/scalar/gpsimd)
            c0 = loaded[0]
            if upto <= c0:
                return
            nc.sync.dma_start(
                out=k8[:, 1024 * c0 : 1024 * upto],
                in_=k8_d[:, 1024 * c0 : 1024 * upto],
            )
            nc.gpsimd.dma_start(
                out=v8[:, 1024 * c0 : 1024 * upto],
                in_=v8_d[:, 1024 * c0 : 1024 * upto],
            )
            loaded[0] = upto

        loaded[0] = 4

        # ---- shared epilogue: recip + scale on DVE, then DMA O^T out
        # directly (host untransposes). No PE involvement at all. ----
        def make_epilogue(st, sO, lrow_t, pre_pe=None):
            def phase1():
                if pre_pe is not None:
                    pre_pe()
                rec = sb.tile([128, 512], F32, tag="rec", bufs=2, name=f"rec{st}")
                nc.vector.reciprocal_approx_fast(out=rec[:], in_=lrow_t[:])
                for dh in range(2):
                    t_ = sb.tile([128, 512], F32, tag="oTf", bufs=4, name=f"oTf{st}{dh}")
                    nc.vector.tensor_tensor(t_[:], sO[dh][:], rec[:], M)
                    nc.sync.dma_start(
                        out=o_d[dh * 128 : (dh + 1) * 128, st * 512 : (st + 1) * 512],
                        in_=t_[:],
                    )

            def phase2():
                pass
            return phase1, phase2

        # ---- fp8 slot (st in {0,1,2}) ----
        def do_slot_fp8(st, epi=None, prefetch_cb=None, fin_prev=None):
            n = NCOUNT[st]
            qx = q8[:, st * 1024 : (st + 1) * 1024].rearrange(
                "p (i x) -> p i x", i=2
            )
            sO = [
                ps.tile([128, 512], F32, tag="sO", bufs=3, name=f"sO{st}{d}")
                for d in range(2)
            ]
            lrow = ps.tile([128, 512], F32, tag="lrow", bufs=1, name=f"lrow{st}")

            def emit_front(t):
                pairs = []
                kc = k8[:, t * 1024 : (t + 1) * 1024].rearrange(
                    "p (i x) -> p i x", i=2
                )
                for pair in range(2):
                    sS = ps.tile([128, 1024], F32, tag="sS", bufs=2, name=f"sS{st}{t}{pair}")
                    for sub in range(2):
                        kt_i = 2 * pair + sub
                        nc.tensor.matmul(
                            sS[:, sub * 512 : (sub + 1) * 512],
                            kc[:, :, 128 * kt_i : 128 * (kt_i + 1)],
                            qx,
                            start=True, stop=True, perf_mode=DR,
                        )
                    p8 = sb.tile([128, 1024], F8, tag="p8", bufs=10, name=f"p8{st}{t}{pair}")
                    nc.scalar.activation(
                        p8[:], sS[:], mybir.ActivationFunctionType.Exp,
                        scale=SCALE, bias=bias_t[:],
                    )
                    pairs.append(p8)
                return pairs

            all_pairs = {}
            # masked (diagonal) items first: their DVE mask latency hides
            # under later items' PE work instead of sitting in the drain
            order = [n - 2, 0, n - 1] + list(range(1, n - 2))

            def emit_lT(j):
                t = order[j]
                for jp01 in range(2):
                    x = all_pairs[t][jp01][:].rearrange("p (i x) -> p i x", i=2)
                    nc.tensor.matmul(
                        lrow[:], onesp, x,
                        start=(j == 0 and jp01 == 0),
                        stop=(j == n - 1 and jp01 == 1),
                        perf_mode=DR,
                    )

            def emit_back(j):
                t = order[j]
                pairs = all_pairs[t]
                if t >= n - 2:
                    pos = t - (n - 2)
                    for pair in range(2):
                        col = st * 8 + pos * 4 + 2 * pair
                        nc.vector.scalar_tensor_tensor(
                            pairs[pair][:],
                            iota[:],
                            thr[:, col : col + 1],
                            pairs[pair][:],
                            mybir.AluOpType.is_ge, M,
                        )
                for jp01 in range(2):
                    jp = 2 * t + jp01
                    x = pairs[jp01][:].rearrange("p (i x) -> p i x", i=2)
                    vv = v8[:, jp * 512 : (jp + 1) * 512].rearrange(
                        "p (i x) -> p i x", i=2
                    )
                    for dh in range(2):
                        nc.tensor.matmul(
                            sO[dh][:], vv[:, :, dh * 128 : (dh + 1) * 128], x,
                            start=(j == 0 and jp01 == 0),
                            stop=(j == n - 1 and jp01 == 1),
                            perf_mode=DR,
                        )
                # lT deferred one item: keeps the bank-clearing first write
                # clear of the previous slot's epilogue
                if j >= 1:
                    emit_lT(j - 1)

            done = [0]
            for pos, t in enumerate(order):
                all_pairs[t] = emit_front(t)
                if pos == 0 and fin_prev is not None:
                    fin_prev[0]()
                if pos == 3 and prefetch_cb is not None:
                    prefetch_cb()
                if pos > 1:
                    emit_back(done[0]); done[0] += 1
                if pos == 1:
                    if fin_prev is not None:
                        fin_prev[1]()
                    if epi is not None:
                        epi[0]()
            while done[0] < n - 2:
                emit_back(done[0]); done[0] += 1

            # last two backs deferred into the next slot so its exps
            # overlap this slot's final PV/lT drain
            def fin1():
                emit_back(done[0]); done[0] += 1

            def fin2():
                emit_back(done[0]); done[0] += 1
                emit_lT(n - 1)
            return (fin1, fin2), make_epilogue(st, sO, lrow)

        # ---- slot 3: f32r path ----
        def do_slot3(epi=None, prefetch_cb=None, fin_prev=None):
            st, n = 3, 2
            sO = [
                ps.tile([128, 512], F32, tag="sO", bufs=3, name=f"sO3{d}")
                for d in range(2)
            ]
            lrow = ps.tile([128, 512], F32, tag="lrow", bufs=1, name="lrow3")
            pSum = sb.tile([128, 512], F32, tag="pSum", bufs=1, name="pSum3")

            def emit_front(t):
                pairs = []
                for pair in range(2):
                    sS = ps.tile([128, 1024], F32, tag="sS", bufs=2, name=f"sS3{t}{pair}")
                    for sub in range(2):
                        kt_i = 2 * pair + sub
                        koff = 512 * t + 128 * kt_i
                        for dt in range(2):
                            nc.tensor.matmul(
                                sS[:, sub * 512 : (sub + 1) * 512],
                                kf3[:, dt * 1024 + koff : dt * 1024 + koff + 128],
                                qf3[:, dt * 512 : (dt + 1) * 512],
                                start=(dt == 0), stop=(dt == 1),
                            )
                    pT = sb.tile([128, 1024], BF16, tag="pT3", bufs=6, name=f"pT3{t}{pair}")
                    nc.scalar.activation(
                        pT[:], sS[:], mybir.ActivationFunctionType.Exp,
                        scale=SCALE, bias=bias_t[:],
                    )
                    pairs.append(pT)
                return pairs

            def emit_back(t, pairs):
                pos = t - (n - 2)
                for pair in range(2):
                    col = st * 8 + pos * 4 + 2 * pair
                    nc.vector.scalar_tensor_tensor(
                        pairs[pair][:],
                        iota[:],
                        thr[:, col : col + 1],
                        pairs[pair][:],
                        mybir.AluOpType.is_ge, M,
                    )
                for kt_i in range(4):
                    pair, sub = kt_i // 2, kt_i % 2
                    g = 4 * t + kt_i
                    x = pairs[pair][:, sub * 512 : (sub + 1) * 512]
                    for dt in range(2):
                        nc.tensor.matmul(
                            sO[dt][:],
                            vf3[:, g * 256 + dt * 128 : g * 256 + (dt + 1) * 128],
                            x,
                            start=(t == 0 and kt_i == 0),
                            stop=(t == n - 1 and kt_i == 3),
                        )
                # denominator partial sums (per-partition)
                f = pairs[0][:]
                g2 = pairs[1][:]
                tmp = sb.tile([128, 512], F32, tag="fold", bufs=2, name=f"fold3{t}")
                nc.vector.tensor_tensor(tmp[:], f[:, 0:512], f[:, 512:1024], A)
                nc.vector.tensor_tensor(tmp[:], tmp[:], g2[:, 0:512], A)
                nc.vector.tensor_tensor(tmp[:], tmp[:], g2[:, 512:1024], A)
                if t == 0:
                    nc.vector.tensor_copy(pSum[:], tmp[:])
                else:
                    nc.vector.tensor_tensor(pSum[:], pSum[:], tmp[:], A)

            hist = {}
            for t in range(n):
                hist[t] = emit_front(t)
                if t == 0 and fin_prev is not None:
                    fin_prev[0]()
                if t == 1 and prefetch_cb is not None:
                    prefetch_cb()
                if t == 1:
                    if fin_prev is not None:
                        fin_prev[1]()
                    if epi is not None:
                        epi[0]()
            pend3 = sorted(hist)

            def fin1():
                emit_back(pend3[0], hist[pend3[0]])

            def fin2():
                emit_back(pend3[1], hist[pend3[1]])
            if epi is not None:
                epi[1]()

            def pre_pe():
                # cross-partition sum via plain f32 ones-matmul (broadcast)
                nc.tensor.matmul(
                    lrow[:], ones_f[:, 0:128], pSum[:], start=True, stop=True
                )
            return (fin1, fin2), make_epilogue(st, sO, lrow, pre_pe=pre_pe)

        # ---- emission ----
        fin, epi = do_slot_fp8(2, prefetch_cb=lambda: load_chunks(6))
        fin, epi = do_slot3(epi=epi, prefetch_cb=lambda: load_chunks(8),
                            fin_prev=fin)
        for st in (1, 0):
            fin, epi = do_slot_fp8(st, epi=epi, fin_prev=fin)
        fin[0]()
        fin[1]()
        epi[0]()
        epi[1]()

    nc.compile()
    return nc


# ---------------- host-side packing ----------------

def make_core_inputs(query, key, value):
    """query/key/value: [B, S, D] f32 numpy. Returns list of 8 in_maps."""
    import ml_dtypes

    f8 = ml_dtypes.float8_e4m3
    kk = np.arange(128, dtype=np.float32)
    in_maps = []
    per_batch = {}
    for b in range(B):
        K8 = key[b].astype(f8)
        V8 = value[b].astype(f8)
        # chunk-interleaved: [p, c*1024 + i*512 + k]
        k8 = np.zeros((128, 8192), dtype=K8.dtype)
        for c in range(8):
            for i in range(2):
                k8[:, c * 1024 + i * 512 : c * 1024 + (i + 1) * 512] = K8[
                    512 * c : 512 * (c + 1), 128 * i : 128 * (i + 1)
                ].T
        v8 = (
            V8.reshape(16, 2, 128, 256).transpose(2, 0, 1, 3).reshape(128, 8192)
        )
        kf3 = np.concatenate(
            [key[b, :1024, :128].T, key[b, :1024, 128:].T], axis=1
        ).astype(ml_dtypes.bfloat16)  # [128, 2048]
        vf3 = (
            value[b, :1024].reshape(8, 128, 256).transpose(1, 0, 2).reshape(128, 2048)
        ).astype(ml_dtypes.bfloat16)
        per_batch[b] = (k8, v8, kf3, vf3)

    for c in range(8):
        b, h = c // 2, c % 2
        blocks = SLOTBLK[h]
        k8, v8, kf3, vf3 = per_batch[b]
        # q8: slots 0..2, transposed pair-packed fp8
        q8 = np.zeros((128, 3072), dtype=np.float32)
        for st in range(3):
            blk = blocks[st]
            Qb = query[b, 512 * blk : 512 * (blk + 1)]  # [512, 256]
            for i in range(2):
                q8[:, st * 1024 + i * 512 : st * 1024 + (i + 1) * 512] = Qb[
                    :, i * 128 : (i + 1) * 128
                ].T
        q8 = q8.astype(ml_dtypes.float8_e4m3)
        # qf3: slot3 block, f32 transposed
        blk3 = blocks[3]
        Q3 = query[b, 512 * blk3 : 512 * (blk3 + 1)]
        qf3 = np.concatenate([Q3[:, :128].T, Q3[:, 128:].T], axis=1).astype(
            ml_dtypes.bfloat16
        )
        thr = np.zeros((128, 32), dtype=np.float32)
        for st in range(4):
            j_abs = blocks[st]
            n = NCOUNT[st]
            for pos in range(2):
                chunk = n - 2 + pos
                for kt in range(4):
                    col = st * 8 + pos * 4 + kt
                    if chunk < j_abs:
                        thr[:, col] = -1e4
                    elif chunk == j_abs:
                        thr[:, col] = 128.0 * kt + kk
                    else:
                        thr[:, col] = 1e4
        in_maps.append(
            {
                "q8": np.ascontiguousarray(q8),
                "k8": np.ascontiguousarray(k8),
                "v8": np.ascontiguousarray(v8),
                "qf3": np.ascontiguousarray(qf3),
                "kf3": np.ascontiguousarray(kf3),
                "vf3": np.ascontiguousarray(vf3),
                "thr": thr,
            }
        )
    return in_maps


def gather_output(results):
    """results: list of 8 dicts with 'o' [NQ, D]. Returns [B, S, D]."""
    out = np.zeros((B, S, D), dtype=np.float32)
    for c in range(8):
        b, h = c // 2, c % 2
        o = results[c]["o"]  # [D, NQ] transposed
        for st, blk in enumerate(SLOTBLK[h]):
            out[b, 512 * blk : 512 * (blk + 1)] = o[:, 512 * st : 512 * (st + 1)].T
    return out


_NC_CACHE = []


def kernel(query, key, value, attention_mask):
    """Full-input causal attention; returns [B, S, D] float32."""
    query = np.ascontiguousarray(np.asarray(query, dtype=np.float32))
    key = np.ascontiguousarray(np.asarray(key, dtype=np.float32))
    value = np.ascontiguousarray(np.asarray(value, dtype=np.float32))
    assert query.shape == (B, S, D) and key.shape == (B, S, D)
    assert value.shape == (B, S, D)
    # attention_mask is all-ones by problem construction (fill: ones).
    if not _NC_CACHE:
        _NC_CACHE.append(build())
    nc = _NC_CACHE[0]
    in_maps = make_core_inputs(query, key, value)
    res = run_bass_kernel_spmd(nc, in_maps, core_ids=list(range(8)))
    return gather_output(res.results)


# revision 31
# speedup vs baseline: 1.1843x; 1.0138x over previous
"""Self-contained TRN2 Bass kernel: causal single-head attention.

B=4, S=4096, D=256, fp32 in/out. 8 NeuronCores, data-parallel:
core c = 2*b + h computes batch b, half h of the query blocks
({7,4,3,0} vs {6,5,2,1}). Mixed precision: slots 0-2 (long prefixes)
use fp8e4 DoubleRow matmuls (QK, PV, and a ones-weight matmul for the
softmax denominator); slot 3 (short prefixes, error-sensitive) uses
f32r. No-max softmax with exp bias -2 so P fits fp8 range; denominator
arrives partition-broadcast as lrow[p,q]=l(q), so normalization is a
plain elementwise multiply before the output transpose.
"""

import sys

for _p in ("/opt/trn_rl_repo", "/root/.axon_site/_ro/trn_rl_repo"):
    if _p not in sys.path:
        sys.path.append(_p)

from contextlib import ExitStack

import numpy as np

import concourse.mybir as mybir
import concourse.tile as tile
from concourse import bacc
from concourse.bass_utils import run_bass_kernel_spmd
from concourse.masks import make_identity

F32 = mybir.dt.float32
F32R = mybir.dt.float32r
F8 = mybir.dt.float8e4
BF16 = mybir.dt.bfloat16
DR = mybir.MatmulPerfMode.DoubleRow
A = mybir.AluOpType.add
M = mybir.AluOpType.mult

B, S, D = 4, 4096, 256
NQ = 2048                 # queries per core
NCOUNT = (8, 6, 4, 2)     # key-chunks per slot
SLOTBLK = [[7, 4, 3, 0], [6, 5, 2, 1]]   # abs q-block per slot, per half
SLOT_ORDER = [3, 2, 1, 0]  # shortest-prefix slot first
SCALE = 1.0 / 16.0         # 1/sqrt(D)
BIAS = -2.0                # exp bias (cancels in normalization)


def build():
    nc = bacc.Bacc("TRN2", target_bir_lowering=False, debug=False)
    q8_d = nc.dram_tensor("q8", [128, 3072], F8, kind="ExternalInput").ap()
    k8_d = nc.dram_tensor("k8", [128, 8192], F8, kind="ExternalInput").ap()
    v8_d = nc.dram_tensor("v8", [128, 8192], F8, kind="ExternalInput").ap()
    qf3_d = nc.dram_tensor("qf3", [128, 1024], BF16, kind="ExternalInput").ap()
    kf3_d = nc.dram_tensor("kf3", [128, 2048], BF16, kind="ExternalInput").ap()
    vf3_d = nc.dram_tensor("vf3", [128, 2048], BF16, kind="ExternalInput").ap()
    thr_d = nc.dram_tensor("thr", [128, 32], F32, kind="ExternalInput").ap()
    o_d = nc.dram_tensor("o", [D, NQ], F32, kind="ExternalOutput").ap()

    with tile.TileContext(nc) as tc, ExitStack() as ctx:
        const = ctx.enter_context(tc.tile_pool(name="const", bufs=1))
        sb = ctx.enter_context(tc.tile_pool(name="sb", bufs=8))
        ps = ctx.enter_context(tc.tile_pool(name="ps", bufs=1, space="PSUM"))

        # ---- PE warmup first: ramp the clock during DMA wait ----
        wsrc = const.tile([128, 128], F32R, name="wsrc")
        nc.vector.memset(wsrc[:].bitcast(F32), 1.0)
        for w in range(4):
            wps = ps.tile([128, 512], F32, tag="lrow", bufs=1, name=f"warm{w}")
            nc.tensor.transpose(wps[:, 0:128].bitcast(F32R), wsrc[:], wsrc[:])

        # ---- constant tiles (filled after DMA issue) ----
        ident_f = const.tile([128, 128], F32, name="ident_f")
        ones_f = const.tile([128, 1024], F32, name="ones_f")
        ones8 = const.tile([128, 1024], F8, name="ones8")
        bias_t = const.tile([128, 1], F32, name="bias_t")
        iota = const.tile([128, 1024], F32, name="iota")
        iota_i = const.tile([128, 1024], mybir.dt.int32, name="iota_i")
        thr = const.tile([128, 32], F32, name="thr")

        q8 = const.tile([128, 3072], F8, name="q8")
        k8 = const.tile([128, 8192], F8, name="k8")
        v8 = const.tile([128, 8192], F8, name="v8")
        qf3 = const.tile([128, 1024], BF16, name="qf3")
        kf3 = const.tile([128, 2048], BF16, name="kf3")
        vf3 = const.tile([128, 2048], BF16, name="vf3")

        onesp = ones8[:].rearrange("p (i x) -> p i x", i=2)[:, :, 0:128]

        # ---- input DMAs. DMA bandwidth is globally shared and served
        # roughly FIFO by issue time, so issue strictly in deadline order
        # (slot2 processes items 2,3 first).
        # processing order within slot2 is items [2, 0, 3, 1]
        # fronts consume k-chunks 2 positions before backs consume the
        # matching v-chunks; interleave deadlines: k2,k0,k3,v2,k1,v0,v3,v1
        nc.gpsimd.dma_start(out=q8[:, 2048:3072], in_=q8_d[:, 2048:3072])
        nc.sync.dma_start(out=k8[:, 2048:3072], in_=k8_d[:, 2048:3072])
        nc.scalar.dma_start(out=thr[:], in_=thr_d)
        nc.scalar.dma_start(out=k8[:, 0:1024], in_=k8_d[:, 0:1024])
        nc.sync.dma_start(out=k8[:, 3072:4096], in_=k8_d[:, 3072:4096])
        nc.gpsimd.dma_start(out=v8[:, 2048:3072], in_=v8_d[:, 2048:3072])
        nc.scalar.dma_start(out=k8[:, 1024:2048], in_=k8_d[:, 1024:2048])
        nc.sync.dma_start(out=v8[:, 0:1024], in_=v8_d[:, 0:1024])
        nc.gpsimd.dma_start(out=v8[:, 3072:4096], in_=v8_d[:, 3072:4096])
        nc.scalar.dma_start(out=v8[:, 1024:2048], in_=v8_d[:, 1024:2048])

        # masks need iota by ~13us: generate before the non-critical issues
        nc.gpsimd.iota(iota_i[:, 0:512], pattern=[[1, 512]], base=0,
                       channel_multiplier=0)
        nc.gpsimd.iota(iota_i[:, 512:1024], pattern=[[1, 512]], base=-128,
                       channel_multiplier=0)
        nc.vector.tensor_copy(iota[:], iota_i[:])

        nc.gpsimd.dma_start(out=qf3[:], in_=qf3_d)
        nc.sync.dma_start(out=kf3[:], in_=kf3_d)
        nc.scalar.dma_start(out=q8[:, 0:2048], in_=q8_d[:, 0:2048])
        nc.gpsimd.dma_start(out=vf3[:], in_=vf3_d)

        # ---- on-chip constants ----
        make_identity(nc, ident_f[:])
        nc.vector.memset(ones_f[:], 1.0)
        nc.vector.tensor_copy(ones8[:], ones_f[:])
        nc.vector.memset(bias_t[:], BIAS)

        loaded = [0]

        def load_chunks(upto, first=False):
            # k8/v8 chunks [loaded, upto); one DMA per tensor-half, spread
            # over the three DMA-capable queues (sync/scalar/gpsimd)
            c0 = loaded[0]
            if upto <= c0:
                return
            nc.sync.dma_start(
                out=k8[:, 1024 * c0 : 1024 * upto],
                in_=k8_d[:, 1024 * c0 : 1024 * upto],
            )
            nc.gpsimd.dma_start(
                out=v8[:, 1024 * c0 : 1024 * upto],
                in_=v8_d[:, 1024 * c0 : 1024 * upto],
            )
            loaded[0] = upto

        loaded[0] = 4

        # ---- shared epilogue: recip + scale on DVE, then DMA O^T out
        # directly (host untransposes). No PE involvement at all. ----
        def make_epilogue(st, sO, lrow_t, pre_pe=None):
            def phase1():
                if pre_pe is not None:
                    pre_pe()
                rec = sb.tile([128, 512], F32, tag="rec", bufs=2, name=f"rec{st}")
                nc.vector.reciprocal_approx_fast(out=rec[:], in_=lrow_t[:])
                for dh in range(2):
                    t_ = sb.tile([128, 512], F32, tag="oTf", bufs=4, name=f"oTf{st}{dh}")
                    nc.vector.tensor_tensor(t_[:], sO[dh][:], rec[:], M)
                    nc.sync.dma_start(
                        out=o_d[dh * 128 : (dh + 1) * 128, st * 512 : (st + 1) * 512],
                        in_=t_[:],
                    )

            def phase2():
                pass
            return phase1, phase2

        # ---- fp8 slot (st in {0,1,2}) ----
        def do_slot_fp8(st, epi=None, prefetch_cb=None, fin_prev=None):
            n = NCOUNT[st]
            qx = q8[:, st * 1024 : (st + 1) * 1024].rearrange(
                "p (i x) -> p i x", i=2
            )
            sO = [
                ps.tile([128, 512], F32, tag="sO", bufs=3, name=f"sO{st}{d}")
                for d in range(2)
            ]
            lrow = ps.tile([128, 512], F32, tag="lrow", bufs=1, name=f"lrow{st}")

            def emit_front(t):
                pairs = []
                kc = k8[:, t * 1024 : (t + 1) * 1024].rearrange(
                    "p (i x) -> p i x", i=2
                )
                for pair in range(2):
                    sS = ps.tile([128, 1024], F32, tag="sS", bufs=2, name=f"sS{st}{t}{pair}")
                    for sub in range(2):
                        kt_i = 2 * pair + sub
                        nc.tensor.matmul(
                            sS[:, sub * 512 : (sub + 1) * 512],
                            kc[:, :, 128 * kt_i : 128 * (kt_i + 1)],
                            qx,
                            start=True, stop=True, perf_mode=DR,
                        )
                    p8 = sb.tile([128, 1024], F8, tag="p8", bufs=10, name=f"p8{st}{t}{pair}")
                    nc.scalar.activation(
                        p8[:], sS[:], mybir.ActivationFunctionType.Exp,
                        scale=SCALE, bias=bias_t[:],
                    )
                    pairs.append(p8)
                return pairs

            all_pairs = {}
            # masked (diagonal) items first: their DVE mask latency hides
            # under later items' PE work instead of sitting in the drain
            order = [n - 2, 0, n - 1] + list(range(1, n - 2))

            def emit_lT(j):
                t = order[j]
                for jp01 in range(2):
                    x = all_pairs[t][jp01][:].rearrange("p (i x) -> p i x", i=2)
                    nc.tensor.matmul(
                        lrow[:], onesp, x,
                        start=(j == 0 and jp01 == 0),
                        stop=(j == n - 1 and jp01 == 1),
                        perf_mode=DR,
                    )

            def emit_back(j):
                t = order[j]
                pairs = all_pairs[t]
                if t >= n - 2:
                    pos = t - (n - 2)
                    for pair in range(2):
                        col = st * 8 + pos * 4 + 2 * pair
                        nc.vector.scalar_tensor_tensor(
                            pairs[pair][:],
                            iota[:],
                            thr[:, col : col + 1],
                            pairs[pair][:],
                            mybir.AluOpType.is_ge, M,
                        )
                for jp01 in range(2):
                    jp = 2 * t + jp01
                    x = pairs[jp01][:].rearrange("p (i x) -> p i x", i=2)
                    vv = v8[:, jp * 512 : (jp + 1) * 512].rearrange(
                        "p (i x) -> p i x", i=2
                    )
                    for dh in range(2):
                        nc.tensor.matmul(
                            sO[dh][:], vv[:, :, dh * 128 : (dh + 1) * 128], x,
                            start=(j == 0 and jp01 == 0),
                            stop=(j == n - 1 and jp01 == 1),
                            perf_mode=DR,
                        )
                # lT deferred one item: keeps the bank-clearing first write
                # clear of the previous slot's epilogue
                if j >= 1:
                    emit_lT(j - 1)

            done = [0]
            for pos, t in enumerate(order):
                all_pairs[t] = emit_front(t)
                if pos == 0 and fin_prev is not None:
                    fin_prev[0]()
                if pos == 3 and prefetch_cb is not None:
                    prefetch_cb()
                if pos > 1:
                    emit_back(done[0]); done[0] += 1
                if pos == 1:
                    if fin_prev is not None:
                        fin_prev[1]()
                    if epi is not None:
                        epi[0]()
            while done[0] < n - 2:
                emit_back(done[0]); done[0] += 1

            # last two backs deferred into the next slot so its exps
            # overlap this slot's final PV/lT drain
            def fin1():
                emit_back(done[0]); done[0] += 1

            def fin2():
                emit_back(done[0]); done[0] += 1
                emit_lT(n - 1)
            return (fin1, fin2), make_epilogue(st, sO, lrow)

        # ---- slot 3: f32r path ----
        def do_slot3(epi=None, prefetch_cb=None, fin_prev=None):
            st, n = 3, 2
            sO = [
                ps.tile([128, 512], F32, tag="sO", bufs=3, name=f"sO3{d}")
                for d in range(2)
            ]
            lrow = ps.tile([128, 512], F32, tag="lrow", bufs=1, name="lrow3")
            pSum = sb.tile([128, 512], F32, tag="pSum", bufs=1, name="pSum3")

            def emit_front(t):
                pairs = []
                for pair in range(2):
                    sS = ps.tile([128, 1024], F32, tag="sS", bufs=2, name=f"sS3{t}{pair}")
                    for sub in range(2):
                        kt_i = 2 * pair + sub
                        koff = 512 * t + 128 * kt_i
                        for dt in range(2):
                            nc.tensor.matmul(
                                sS[:, sub * 512 : (sub + 1) * 512],
                                kf3[:, dt * 1024 + koff : dt * 1024 + koff + 128],
                                qf3[:, dt * 512 : (dt + 1) * 512],
                                start=(dt == 0), stop=(dt == 1),
                            )
                    pT = sb.tile([128, 1024], BF16, tag="pT3", bufs=6, name=f"pT3{t}{pair}")
                    nc.scalar.activation(
                        pT[:], sS[:], mybir.ActivationFunctionType.Exp,
                        scale=SCALE, bias=bias_t[:],
                    )
                    pairs.append(pT)
                return pairs

            def emit_back(t, pairs):
                pos = t - (n - 2)
                for pair in range(2):
                    col = st * 8 + pos * 4 + 2 * pair
                    nc.vector.scalar_tensor_tensor(
                        pairs[pair][:],
                        iota[:],
                        thr[:, col : col + 1],
                        pairs[pair][:],
                        mybir.AluOpType.is_ge, M,
                    )
                for kt_i in range(4):
                    pair, sub = kt_i // 2, kt_i % 2
                    g = 4 * t + kt_i
                    x = pairs[pair][:, sub * 512 : (sub + 1) * 512]
                    for dt in range(2):
                        nc.tensor.matmul(
                            sO[dt][:],
                            vf3[:, g * 256 + dt * 128 : g * 256 + (dt + 1) * 128],
                            x,
                            start=(t == 0 and kt_i == 0),
                            stop=(t == n - 1 and kt_i == 3),
                        )
                # denominator partial sums (per-partition)
                f = pairs[0][:]
                g2 = pairs[1][:]
                tmp = sb.tile([128, 512], F32, tag="fold", bufs=2, name=f"fold3{t}")
                nc.vector.tensor_tensor(tmp[:], f[:, 0:512], f[:, 512:1024], A)
                nc.vector.tensor_tensor(tmp[:], tmp[:], g2[:, 0:512], A)
                nc.vector.tensor_tensor(tmp[:], tmp[:], g2[:, 512:1024], A)
                if t == 0:
                    nc.vector.tensor_copy(pSum[:], tmp[:])
                else:
                    nc.vector.tensor_tensor(pSum[:], pSum[:], tmp[:], A)

            hist = {}
            for t in range(n):
                hist[t] = emit_front(t)
                if t == 0 and fin_prev is not None:
                    fin_prev[0]()
                if t == 1 and prefetch_cb is not None:
                    prefetch_cb()
                if t == 1:
                    if fin_prev is not None:
                        fin_prev[1]()
                    if epi is not None:
                        epi[0]()
            pend3 = sorted(hist)

            def fin1():
                emit_back(pend3[0], hist[pend3[0]])

            def fin2():
                emit_back(pend3[1], hist[pend3[1]])
            if epi is not None:
                epi[1]()

            def pre_pe():
                # cross-partition sum via plain f32 ones-matmul (broadcast)
                nc.tensor.matmul(
                    lrow[:], ones_f[:, 0:128], pSum[:], start=True, stop=True
                )
            return (fin1, fin2), make_epilogue(st, sO, lrow, pre_pe=pre_pe)

        # ---- emission ----
        fin, epi = do_slot_fp8(2, prefetch_cb=lambda: load_chunks(6))
        fin, epi = do_slot3(epi=epi, prefetch_cb=lambda: load_chunks(8),
                            fin_prev=fin)
        for st in (1, 0):
            fin, epi = do_slot_fp8(st, epi=epi, fin_prev=fin)
        fin[0]()
        fin[1]()
        epi[0]()
        epi[1]()

    nc.compile()
    return nc


# ---------------- host-side packing ----------------

def make_core_inputs(query, key, value):
    """query/key/value: [B, S, D] f32 numpy. Returns list of 8 in_maps."""
    import ml_dtypes

    f8 = ml_dtypes.float8_e4m3
    kk = np.arange(128, dtype=np.float32)
    in_maps = []
    per_batch = {}
    for b in range(B):
        K8 = key[b].astype(f8)
        V8 = value[b].astype(f8)
        # chunk-interleaved: [p, c*1024 + i*512 + k]
        k8 = np.zeros((128, 8192), dtype=K8.dtype)
        for c in range(8):
            for i in range(2):
                k8[:, c * 1024 + i * 512 : c * 1024 + (i + 1) * 512] = K8[
                    512 * c : 512 * (c + 1), 128 * i : 128 * (i + 1)
                ].T
        v8 = (
            V8.reshape(16, 2, 128, 256).transpose(2, 0, 1, 3).reshape(128, 8192)
        )
        kf3 = np.concatenate(
            [key[b, :1024, :128].T, key[b, :1024, 128:].T], axis=1
        ).astype(ml_dtypes.bfloat16)  # [128, 2048]
        vf3 = (
            value[b, :1024].reshape(8, 128, 256).transpose(1, 0, 2).reshape(128, 2048)
        ).astype(ml_dtypes.bfloat16)
        per_batch[b] = (k8, v8, kf3, vf3)

    for c in range(8):
        b, h = c // 2, c % 2
        blocks = SLOTBLK[h]
        k8, v8, kf3, vf3 = per_batch[b]
        # q8: slots 0..2, transposed pair-packed fp8
        q8 = np.zeros((128, 3072), dtype=np.float32)
        for st in range(3):
            blk = blocks[st]
            Qb = query[b, 512 * blk : 512 * (blk + 1)]  # [512, 256]
            for i in range(2):
                q8[:, st * 1024 + i * 512 : st * 1024 + (i + 1) * 512] = Qb[
                    :, i * 128 : (i + 1) * 128
                ].T
        q8 = q8.astype(ml_dtypes.float8_e4m3)
        # qf3: slot3 block, f32 transposed
        blk3 = blocks[3]
        Q3 = query[b, 512 * blk3 : 512 * (blk3 + 1)]
        qf3 = np.concatenate([Q3[:, :128].T, Q3[:, 128:].T], axis=1).astype(
            ml_dtypes.bfloat16
        )
        thr = np.zeros((128, 32), dtype=np.float32)
        for st in range(4):
            j_abs = blocks[st]
            n = NCOUNT[st]
            for pos in range(2):
                chunk = n - 2 + pos
                for kt in range(4):
                    col = st * 8 + pos * 4 + kt
                    if chunk < j_abs:
                        thr[:, col] = -1e4
                    elif chunk == j_abs:
                        thr[:, col] = 128.0 * kt + kk
                    else:
                        thr[:, col] = 1e4
        in_maps.append(
            {
                "q8": np.ascontiguousarray(q8),
                "k8": np.ascontiguousarray(k8),
                "v8": np.ascontiguousarray(v8),
                "qf3": np.ascontiguousarray(qf3),
                "kf3": np.ascontiguousarray(kf3),
                "vf3": np.ascontiguousarray(vf3),
                "thr": thr,
            }
        )
    return in_maps


def gather_output(results):
    """results: list of 8 dicts with 'o' [NQ, D]. Returns [B, S, D]."""
    out = np.zeros((B, S, D), dtype=np.float32)
    for c in range(8):
        b, h = c // 2, c % 2
        o = results[c]["o"]  # [D, NQ] transposed
        for st, blk in enumerate(SLOTBLK[h]):
            out[b, 512 * blk : 512 * (blk + 1)] = o[:, 512 * st : 512 * (st + 1)].T
    return out


_NC_CACHE = []


def kernel(query, key, value, attention_mask):
    """Full-input causal attention; returns [B, S, D] float32."""
    query = np.ascontiguousarray(np.asarray(query, dtype=np.float32))
    key = np.ascontiguousarray(np.asarray(key, dtype=np.float32))
    value = np.ascontiguousarray(np.asarray(value, dtype=np.float32))
    assert query.shape == (B, S, D) and key.shape == (B, S, D)
    assert value.shape == (B, S, D)
    # attention_mask is all-ones by problem construction (fill: ones).
    if not _NC_CACHE:
        _NC_CACHE.append(build())
    nc = _NC_CACHE[0]
    in_maps = make_core_inputs(query, key, value)
    res = run_bass_kernel_spmd(nc, in_maps, core_ids=list(range(8)))
    return gather_output(res.results)
